# revision 7
# baseline (speedup 1.0000x reference)
"""Trainium2 Bass kernel for nn_AlignmentMatrix (fp8 e3m4 edition).

Math: out[b,i,j] = ctx[b,i,:]@w1 + asp[b,j,:]@w2 + (ctx[b,i,:]*w3)@asp[b,j,:]
where w_u = cat([w1,w2,w3]).

Device computes out.T[b][j,i] = sum_k M[b][k,j] * X[b][k,i], contraction
split into chunks of 128,128,128,128,96 rows.  The last chunk holds the
88 remaining ctx rows plus 4 correction rows carrying the exact rank-2
term asp_term[j] + ctx_term[i] as fp8 hi/lo pairs (t = 8*e3m4(t/8) +
e3m4(t - 8*e3m4(t/8))):
    lhsT rows 88..91: [asp_hi, asp_lo, 8.0, 1.0]
    rhs  rows 88..91: [8.0,    1.0,    ctx_hi, ctx_lo]
rows 92..95 are zero padding so the granule partition count (96) keeps
all 16 SDMA engines engaged (the HWDGE splits a granule's partitions
evenly across engines: counts must be divisible by 16 or only
gcd(P, 16) engines carry the stream).

All streamed data is fp8 e3m4 (4 mantissa bits; measured end-to-end rel
err ~0.011 vs the 2e-2 gate), halving HBM traffic vs bf16.  PE runs in
128x32 column-tiling mode: the 4 batches of a group map to PE column
tiles (0,0),(0,32),(0,64),(0,96) writing the four 32-partition quadrants
of one PSUM bank, so a 4-batch round of FD=512 matmuls takes ~one matmul
time (~230ns warm).  Each input granule gets its OWN semaphore: a shared
cumulative semaphore can hit threshold k while a lagging engine still
streams granule k-1 (observed as flaky NaN).  Outputs ride the scalar
HWDGE ring and overlap the input stream on the sync ring.
"""

import numpy as np
import ml_dtypes

# Problem shape (hardcoded per spec)
B, L1, L2, D = 64, 512, 32, 600
NCORES = 8
NB = B // NCORES          # batches per core (8)
NCH = 5                   # contraction chunks: 128*4 + 96
CROWS = (128, 128, 128, 128, 96)
KTAIL = 88                # real ctx rows in the last chunk
NG = 2                    # batch groups per core
GB = NB // NG             # batches per group (4)
MLEN = NB * NCH * L2      # 1280 m-block bytes per partition
GW = GB * L1              # 2048 ctx granule width (4 batches)
FREE = MLEN + NG * NCH * GW   # 21760 total free bytes per partition
F8 = ml_dtypes.float8_e3m4
F8MAX = 15.5

_CACHE = {}


def _ensure_profile_hook():
    """Register the NTFF profile hook so run(trace=True) works under axon."""
    import sys, types
    if 'antenv.axon_hooks' in sys.modules:
        return
    try:
        from trn_agent_boot.trn_boot import _ntff_profile_via_ctypes
        hook = _ntff_profile_via_ctypes('/opt/axon/libaxon_pjrt.so')
        mod = types.ModuleType('antenv.axon_hooks')
        mod.get_axon_ntff_profile_hook = lambda: hook
        sys.modules['antenv.axon_hooks'] = mod
    except Exception:
        pass


def _build_nc():
    """Build the per-core Bass graph (identical SPMD program for all 8 cores)."""
    import contextlib
    import concourse.bass as bass
    import concourse.mybir as mybir

    fp8 = mybir.dt.float8e3
    bf16 = mybir.dt.bfloat16
    f32 = mybir.dt.float32

    nc = bass.Bass()

    big_ext = nc.declare_dram_parameter("big", [128, FREE], fp8, isOutput=False)
    # Device out layout: [p = 32*(b%4) + j, (b//4)*512 + i]; host decodes.
    out_ext = nc.declare_dram_parameter("out", [128, NG * L1], bf16, isOutput=True)

    def moff(b, c):
        return (b * NCH + c) * L2

    def xoff(g, c):
        return MLEN + (g * NCH + c) * GW

    # Input granules: (rows, start, end).  The m+corr block is merged with
    # the first ctx granule (they are contiguous) so no per-engine semaphore
    # bubble separates them; then per batch-group: chunk-pair granules
    # c01, c23 and the 96-row c4 tail.
    granules = [(128, 0, MLEN + 2 * GW)]
    for g in range(NG):
        base = xoff(g, 0)
        if g > 0:
            granules.append((128, base, base + 2 * GW))
        granules.append((128, base + 2 * GW, base + 4 * GW))
        granules.append((96, base + 4 * GW, base + 5 * GW))
    NDMA = len(granules)   # 6

    with contextlib.ExitStack() as ctx:
        big_sb = ctx.enter_context(nc.sbuf_tensor("big_sb", [128, FREE], fp8))
        out_sb = ctx.enter_context(nc.sbuf_tensor("out_sb", [128, NG * L1], bf16))
        psums = [
            ctx.enter_context(nc.psum_tensor(f"pg{g}", [128, L1], f32))
            for g in range(NG)
        ]
        ps_dummy = ctx.enter_context(nc.psum_tensor("ps_dummy", [L2, L1], f32))
        in_sems = [
            ctx.enter_context(nc.semaphore(f"in{k}")) for k in range(NDMA)
        ]
        mm_sem = ctx.enter_context(nc.semaphore("mm_sem"))
        cpv = ctx.enter_context(nc.semaphore("cpv"))
        cps = ctx.enter_context(nc.semaphore("cps"))
        odma = ctx.enter_context(nc.semaphore("odma"))
        block = ctx.enter_context(nc.Block(no_gpsimd_drain=True))

        @block.sync
        def _(sync):
            for k, (rows, a, b) in enumerate(granules):
                sync.dma_start(
                    big_sb[0:rows, a:b], big_ext[0:rows, a:b]
                ).then_inc(in_sems[k], 16)

        def warm(tensor, n):
            # Dummy matmuls into a dedicated PSUM bank warm the PE HAM clock
            # gate while the first input granules stream in.
            for _ in range(n):
                tensor.matmul(
                    ps_dummy[:],
                    big_sb[0:128, 0:L2],
                    big_sb[0:128, MLEN:MLEN + L1],
                    start=True,
                    stop=True,
                    tile_position=(0, 0),
                )

        @block.tensor
        def _(tensor):
            warm(tensor, 9)
            for g in range(NG):
                for ci, cs in enumerate(((0, 1), (2, 3), (4,))):
                    if g or ci:
                        # Keep the HAM clock gate warm across the DMA wait so
                        # the tail rounds run at 2.4 GHz, not the cold 1.2.
                        warm(tensor, 2)
                    tensor.wait_ge(in_sems[3 * g + ci], 16)
                    for c in cs:
                        rows = CROWS[c]
                        for t in range(GB):
                            b = GB * g + t
                            mm = tensor.matmul(
                                psums[g][32 * t:32 * t + 32, :],
                                big_sb[0:rows, moff(b, c):moff(b, c) + L2],
                                big_sb[0:rows, xoff(g, c) + t * L1:xoff(g, c) + (t + 1) * L1],
                                start=(c == 0),
                                stop=(c == NCH - 1),
                                tile_position=(0, 32 * t),
                            )
                            if c == NCH - 1 and t == GB - 1:
                                mm.then_inc(mm_sem, 1)

        # PSUM evacuation: group 0 (hidden behind group 1's input stream) is
        # one DVE copy + one SWDGE dma.  Group 1 is the critical tail: DVE
        # and ACT each copy half, then the two output halves go out on the
        # scalar-HWDGE and gpsimd-SWDGE rings in parallel.
        H = L1 // 2

        @block.vector
        def _(vector):
            vector.wait_ge(mm_sem, 1)
            vector.tensor_copy(out_sb[:, 0:L1], psums[0][:]).then_inc(cpv, 1)
            vector.wait_ge(mm_sem, 2)
            vector.tensor_copy(
                out_sb[:, L1:L1 + H], psums[1][:, 0:H]
            ).then_inc(cpv, 1)

        @block.scalar
        def _(scalar):
            scalar.wait_ge(mm_sem, 2)
            scalar.copy(
                out_sb[:, L1 + H:2 * L1], psums[1][:, H:L1]
            ).then_inc(cps, 1)
            scalar.wait_ge(cpv, 2)
            scalar.dma_start(
                out_ext[:, L1:L1 + H], out_sb[:, L1:L1 + H]
            ).then_inc(odma, 16)
            scalar.wait_ge(odma, 48)

        @block.gpsimd
        def _(gpsimd):
            gpsimd.wait_ge(cpv, 1)
            gpsimd.dma_start(out_ext[:, 0:L1], out_sb[:, 0:L1]).then_inc(odma, 16)
            gpsimd.wait_ge(cps, 1)
            gpsimd.dma_start(
                out_ext[:, L1 + H:2 * L1], out_sb[:, L1 + H:2 * L1]
            ).then_inc(odma, 16)

    nc.finalize()
    return nc


def _get_nc():
    if 'nc' not in _CACHE:
        _CACHE['nc'] = _build_nc()
    return _CACHE['nc']


def _q8(x):
    return np.clip(x, -F8MAX, F8MAX).astype(F8)


def _hilo(t):
    """t ~= 8*hi + lo with hi, lo both e3m4 (t in roughly +-124)."""
    hi = _q8(t / 8.0)
    lo = _q8(t - 8.0 * hi.astype(np.float32))
    return hi, lo


def _prepare_in_maps(ctx, asp, w_u):
    ctx = np.asarray(ctx, dtype=np.float32)
    asp = np.asarray(asp, dtype=np.float32)
    w = np.asarray(w_u, dtype=np.float32).reshape(-1)
    w1, w2, w3 = w[:D], w[D:2 * D], w[2 * D:]

    big = np.zeros((NCORES, 128, FREE), dtype=F8)

    # m block: [core, p, (b, c, j)]; m[b] = (w3 * asp[b]).T  [600, 32]
    m_q = _q8(asp.transpose(0, 2, 1) * w3[None, :, None])       # [B, 600, 32]
    bm = big[:, :, :MLEN].reshape(NCORES, 128, NB, NCH, L2)
    bm[:, :, :, :4] = m_q[:, :512].reshape(NCORES, NB, 4, 128, L2).transpose(
        0, 3, 1, 2, 4)
    bm[:, :KTAIL, :, 4] = m_q[:, 512:].reshape(NCORES, NB, KTAIL, L2).transpose(
        0, 2, 1, 3)
    at_hi, at_lo = _hilo(asp @ w2)                              # [B, 32]
    bm[:, KTAIL + 0, :, 4] = at_hi.reshape(NCORES, NB, L2)
    bm[:, KTAIL + 1, :, 4] = at_lo.reshape(NCORES, NB, L2)
    bm[:, KTAIL + 2, :, 4] = 8.0
    bm[:, KTAIL + 3, :, 4] = 1.0

    # ctx block: [core, p, (g, c, b4, i)]
    ctx_q = _q8(ctx)                                            # [B, 512, 600]
    bx = big[:, :, MLEN:].reshape(NCORES, 128, NG, NCH, GB, L1)
    bx[:, :, :, :4] = ctx_q[:, :, :512].reshape(
        NCORES, NG, GB, L1, 4, 128).transpose(0, 5, 1, 4, 2, 3)
    bx[:, :KTAIL, :, 4] = ctx_q[:, :, 512:].reshape(
        NCORES, NG, GB, L1, KTAIL).transpose(0, 4, 1, 2, 3)
    ct_hi, ct_lo = _hilo(ctx @ w1)                              # [B, 512]
    bx[:, KTAIL + 0, :, 4] = 8.0
    bx[:, KTAIL + 1, :, 4] = 1.0
    bx[:, KTAIL + 2, :, 4] = ct_hi.reshape(NCORES, NG, GB, L1)
    bx[:, KTAIL + 3, :, 4] = ct_lo.reshape(NCORES, NG, GB, L1)

    return [{"big": np.ascontiguousarray(big[i])} for i in range(NCORES)]


def run(inputs, trace=False, trace_kwargs=None):
    """Run the kernel on the full inputs; returns (out, BassKernelResults)."""
    from concourse import bass_utils
    from concourse.bass_utils import run_bass_kernel_spmd

    if trace:
        _ensure_profile_hook()
        bass_utils.upload_artifacts = lambda tmpdir: tmpdir

    in_maps = _prepare_in_maps(inputs["ctx"], inputs["asp"], inputs["w_u"])
    nc = _get_nc()
    res = run_bass_kernel_spmd(
        nc, in_maps, core_ids=list(range(NCORES)), trace=trace,
        **(trace_kwargs or {}),
    )
    # Gather: device out [p = 32*(b%4) + j, g*512 + i] bf16 -> out[b, i, j].
    outs = []
    for i in range(NCORES):
        arr = np.asarray(res.results[i]["out"]).astype(np.float32)
        arr = arr.reshape(GB, L2, NG, L1)            # [t, j, g, i]
        outs.append(arr.transpose(2, 0, 3, 1).reshape(NB, L1, L2))
    return np.concatenate(outs, axis=0), res


def kernel(batch_size, ctx, asp, w_u):
    inputs = {"ctx": ctx, "asp": asp, "w_u": w_u}
    out, _ = run(inputs)
    if not np.isfinite(out).all():
        # Rare transient device glitch: retry once.
        out, _ = run(inputs)
    return out
